# revision 6
# baseline (speedup 1.0000x reference)
"""HGNN+ (2x HGNNPConv) Trainium2 kernel, 8-core SPMD.

Strategy: the hypergraph v2v mean aggregation is a linear operator
    v2v(X) = Dv^-1 H De^-1 H^T X
with H the [N, E] incidence-count matrix. We materialize H on the host
(pure index preprocessing of pair_v/pair_e), fold the 1/De and 1/Dv
normalizations into column-scaled variants A = H/De (per edge) and
B = (H/Dv)^T (per vertex), shard vertices across the 8 cores, and run
the whole network as a chain of dense matmuls on the TensorEngine:

  per core l (NL = N/8 = 2048 local vertices):
    M1  = relu(Xl @ W1 + b1)            [NL, CH]    (fp16 in, fp32 psum)
    E1p = A_l^T @ M1                    [E, CH]     partial over local vertices
    E1  = AllReduce(E1p)                [E, CH]     (fp16)
    V1t = relu(E1^T-contract B_l)       [CH, NL]    = relu((B_l^T E1)^T)
    M2  = relu(V1^T @ W2 + b2)          [NL, COUT]
    E2p = A_l^T @ M2 ; E2 = AllReduce   [E, COUT]
    OUT = B_l^T @ E2                    [NL, COUT]  (fp32 out)

Layer 1 is pipelined over the two 512-wide channel halves: steps 1+2
for half c0, AllReduce(c0) in flight while steps 1+2 run for c1, and
step 3's first output rows (which only read E1[:, c0]) overlap
AllReduce(c1). Layer 2 chunks its AllReduce over edge-row halves.

Biases are folded in exactly via an augmented contraction row (row 1024
of the padded weight matrices is the bias; the matching activation row
is constant 1), so the kernel is correct for any b1/b2.

All matmul operands are fp16 (PE runs fp16 at full rate; psum
accumulation is fp32), which keeps the error ~5e-4 vs the fp32
reference. Layout choices keep every DMA's innermost run >= 1KB and
avoid all on-device transposes. Bulk streams (A, B) ride the ACT HWDGE
ring; latency-critical loads ride the Sync ring.
"""

import numpy as np

import concourse.bass as bass  # noqa: F401  (bass types used via bacc)
import concourse.mybir as mybir
import concourse.tile as tile
from concourse import bacc
from concourse.bass_utils import run_bass_kernel_spmd

# Problem shapes (hardcoded per spec nn_HGNNP_33629593927812)
N, E, CIN, CH, COUT = 16384, 2048, 1024, 1024, 512
NC = 8                # cores
NL = N // NC          # 2048 local vertices per core
P = 128
KA = CIN // P + 1     # 9 contraction tiles for the augmented (bias) matmuls
MT = NL // P          # 16 local-vertex tiles
ET = E // P           # 16 edge tiles
CHT = CH // 512       # 2 channel halves of the hidden dim
ARC = 2               # layer-2 AllReduce chunks (over edge rows)
EPC = ET // ARC       # edge tiles per layer-2 AllReduce chunk

F16 = mybir.dt.float16
F32 = mybir.dt.float32
RELU = mybir.ActivationFunctionType.Relu

_CACHE: dict = {}


def _build():
    """Build the per-core Bass program (identical on all 8 cores)."""
    nc = bacc.Bacc(None, target_bir_lowering=False, num_devices=NC)

    # Per-core inputs (host-prepared layouts; see kernel() below)
    xt = nc.dram_tensor("xt", [MT, P, KA * P], F16, kind="ExternalInput")
    w1 = nc.dram_tensor("w1", [KA * P, CH], F16, kind="ExternalInput")
    w2 = nc.dram_tensor("w2", [KA * P, COUT], F16, kind="ExternalInput")
    a_t = nc.dram_tensor("a_t", [ET, P, NL], F16, kind="ExternalInput")
    b_t = nc.dram_tensor("b_t", [E, NL], F16, kind="ExternalInput")
    out = nc.dram_tensor("out", [NL, COUT], F32, kind="ExternalOutput")

    RG = [list(range(NC))]

    with tile.TileContext(nc) as tc:
        with (
            tc.tile_pool(name="persist", bufs=1) as persist,
            tc.tile_pool(name="stream", bufs=3) as stream,
            tc.tile_pool(name="stage", bufs=6) as stage,
            tc.tile_pool(name="psum", bufs=6, space="PSUM") as psum_pool,
            tc.tile_pool(name="dram", bufs=1, space="DRAM") as dram,
        ):
            # ---- resident weights (sync ring: needed immediately) ----
            w1_sb = persist.tile([P, KA, CH], F16, tag="slot_w")  # 2.25MB
            nc.sync.dma_start(w1_sb[:], w1.rearrange("(k pi) c -> pi k c", pi=P))
            w2_sb = persist.tile([P, KA, COUT], F16)  # 1.125MB

            # B_l resident; chunk loads are interleaved into step 2's stream
            # (B is only needed at step 3 — loading it up front starves the
            # startup-critical w1/xt transfers of SDMA bandwidth)
            b_sb = persist.tile([P, ET, NL], F16)  # 8MB
            b_v = b_t.rearrange("(po pi) v -> pi po v", pi=P)

            # AllReduce bounce buffers (collectives need internal DRAM tiles;
            # Shared output tiles are single-writer => one tile per chunk)
            e1p_d = [
                dram.tile([E, 512], F16, name=f"e1p_{n}") for n in range(CHT)
            ]
            e1r_d = [
                dram.tile([E, 512], F16, addr_space="Shared", name=f"e1r_{n}")
                for n in range(CHT)
            ]
            e2p_d = [
                dram.tile([E, 256], F16, name=f"e2p_{h}") for h in range(2)
            ]
            e2r_d = [
                dram.tile([E, 256], F16, addr_space="Shared", name=f"e2r_{h}")
                for h in range(2)
            ]
            e1p_v = [t.rearrange("(po pi) c -> pi po c", pi=P) for t in e1p_d]
            e2p_v = [t.rearrange("(po pi) c -> pi po c", pi=P) for t in e2p_d]

            m1_sb = persist.tile([P, MT, CH], F16, tag="slot_a")  # 4MB
            e1_sb = persist.tile([P, ET, CH], F16, tag="slot_e")  # 4MB

            # ---- layer 1, pipelined over channel halves ----
            for n in range(CHT):
                cs = slice(n * 512, (n + 1) * 512)
                # step 1: M1[:, cs] = relu(X @ W1 + b1)[:, cs]
                for m in range(MT):
                    xt_sb = stream.tile([P, KA * P], F16, tag="stream")
                    nc.sync.dma_start(xt_sb[:], xt[m])
                    ps = psum_pool.tile([P, 512], F32, tag="ps")
                    for k in range(KA):
                        nc.tensor.matmul(
                            ps[:],
                            xt_sb[:, k * P:(k + 1) * P],
                            w1_sb[:, k, cs],
                            start=(k == 0),
                            stop=(k == KA - 1),
                        )
                    nc.scalar.activation(m1_sb[:, m, cs], ps[:], RELU)
                # step 2: E1p[:, cs] = A^T M1[:, cs]
                for me in range(ET):
                    a_sb = stream.tile([P, NL], F16, tag="stream")
                    nc.scalar.dma_start(a_sb[:], a_t[me])
                    if n == 0:
                        # sneak one B chunk load in behind each A tile
                        nc.scalar.dma_start(b_sb[:, me, :], b_v[:, me, :])
                    ps = psum_pool.tile([P, 512], F32, tag="ps")
                    for k in range(MT):
                        nc.tensor.matmul(
                            ps[:],
                            a_sb[:, k * P:(k + 1) * P],
                            m1_sb[:, k, cs],
                            start=(k == 0),
                            stop=(k == MT - 1),
                        )
                    st = stage.tile([P, 512], F16, tag="stage")
                    nc.vector.tensor_copy(st[:], ps[:])
                    nc.sync.dma_start(e1p_v[n][:, me, :], st[:])
                nc.gpsimd.collective_compute(
                    "AllReduce",
                    mybir.AluOpType.add,
                    replica_groups=RG,
                    ins=[e1p_d[n].opt()],
                    outs=[e1r_d[n].opt()],
                )

            # E1 back to SBUF as lhsT tiles [e_pi, e_po, c]. These loads wait
            # on the AllReduces, so they are issued only after both collective
            # triggers (a waiting DMA head-of-line-blocks its HWDGE ring).
            for n in range(CHT):
                nc.sync.dma_start(
                    e1_sb[:, :, n * 512:(n + 1) * 512],
                    e1r_d[n].rearrange("(po pi) c -> pi po c", pi=P),
                )
            nc.sync.dma_start(w2_sb[:], w2.rearrange("(k pi) c -> pi k c", pi=P))

            # ---- step 3: V1t = relu(sum_e E1[e,c] B[e,v]), [ch, vl] ----
            # output row-block mc only reads E1[:, mc*128:...] => the first
            # half overlaps AllReduce(c1)
            v1t_sb = persist.tile([P, KA, NL], F16, tag="slot_a")  # 4.5MB
            nc.vector.memset(v1t_sb[:, KA - 1, :], 0.0)
            nc.vector.memset(v1t_sb[0:1, KA - 1, :], 1.0)  # bias row (ch==CH)
            for mc in range(CH // P):
                for nv in range(NL // 512):
                    ps = psum_pool.tile([P, 512], F32, tag="ps")
                    for k in range(ET):
                        nc.tensor.matmul(
                            ps[:],
                            e1_sb[:, k, mc * P:(mc + 1) * P],
                            b_sb[:, k, nv * 512:(nv + 1) * 512],
                            start=(k == 0),
                            stop=(k == ET - 1),
                        )
                    nc.scalar.activation(
                        v1t_sb[:, mc, nv * 512:(nv + 1) * 512], ps[:], RELU
                    )

            # ---- layer 2, pipelined over 256-wide channel halves ----
            # step 4: M2 = relu(V1 @ W2 + b2); step 5: E2p = A^T M2; the
            # AllReduce of half c0 overlaps step 4+5 of half c1, and the
            # AllReduce of c1 overlaps step 6's first half.
            m2_sb = persist.tile([P, MT, COUT], F16, tag="slot_w")  # 2MB
            for n2 in range(2):
                c2 = slice(n2 * 256, (n2 + 1) * 256)
                for m in range(MT):
                    ps = psum_pool.tile([P, 512], F32, tag="ps")
                    for k in range(KA):
                        nc.tensor.matmul(
                            ps[:, :256],
                            v1t_sb[:, k, m * P:(m + 1) * P],
                            w2_sb[:, k, c2],
                            start=(k == 0),
                            stop=(k == KA - 1),
                        )
                    nc.scalar.activation(m2_sb[:, m, c2], ps[:, :256], RELU)
                for me in range(ET):
                    a_sb = stream.tile([P, NL], F16, tag="stream")
                    nc.scalar.dma_start(a_sb[:], a_t[me])
                    ps = psum_pool.tile([P, 512], F32, tag="ps")
                    for k in range(MT):
                        nc.tensor.matmul(
                            ps[:, :256],
                            a_sb[:, k * P:(k + 1) * P],
                            m2_sb[:, k, c2],
                            start=(k == 0),
                            stop=(k == MT - 1),
                        )
                    st = stage.tile([P, 512], F16, tag="stage")
                    nc.vector.tensor_copy(st[:, :256], ps[:, :256])
                    nc.sync.dma_start(e2p_v[n2][:, me, :], st[:, :256])
                nc.gpsimd.collective_compute(
                    "AllReduce",
                    mybir.AluOpType.add,
                    replica_groups=RG,
                    ins=[e2p_d[n2].opt()],
                    outs=[e2r_d[n2].opt()],
                )

            e2_sb = persist.tile([P, ET, COUT], F16, tag="slot_e")  # 2MB
            for n2 in range(2):
                nc.sync.dma_start(
                    e2_sb[:, :, n2 * 256:(n2 + 1) * 256],
                    e2r_d[n2].rearrange("(po pi) c -> pi po c", pi=P),
                )

            # ---- step 6: OUT = sum_e B[e,v] E2[e,c2], [vl, c2] ----
            out_v = out.rearrange("(po pi) c -> pi po c", pi=P)
            for n2 in range(2):
                c2 = slice(n2 * 256, (n2 + 1) * 256)
                for m in range(MT):
                    ps = psum_pool.tile([P, 512], F32, tag="ps")
                    for k in range(ET):
                        nc.tensor.matmul(
                            ps[:, :256],
                            b_sb[:, k, m * P:(m + 1) * P],
                            e2_sb[:, k, c2],
                            start=(k == 0),
                            stop=(k == ET - 1),
                        )
                    st = stage.tile([P, 512], F32, tag="stage_out")
                    nc.vector.tensor_copy(st[:, :256], ps[:, :256])
                    nc.sync.dma_start(out_v[:, m, c2], st[:, :256])

    nc.compile()
    return nc


def _prepare_inputs(feature_hyg, pair_v, pair_e, W1, b1, W2, b2):
    X = np.ascontiguousarray(np.asarray(feature_hyg, dtype=np.float32))
    pv = np.asarray(pair_v).astype(np.int64)
    pe = np.asarray(pair_e).astype(np.int64)
    W1 = np.asarray(W1, dtype=np.float32)
    b1 = np.asarray(b1, dtype=np.float32)
    W2 = np.asarray(W2, dtype=np.float32)
    b2 = np.asarray(b2, dtype=np.float32)

    ec = np.bincount(pe, minlength=E).astype(np.float32)
    vc = np.bincount(pv, minlength=N).astype(np.float32)
    H = (
        np.bincount(pv * E + pe, minlength=N * E)
        .astype(np.float32)
        .reshape(N, E)
    )
    A = H / np.maximum(ec, 1.0)[None, :]          # [N, E], col-scaled by 1/De
    Bm = (H / np.maximum(vc, 1.0)[:, None]).T     # [E, N], col-scaled by 1/Dv

    W1a = np.zeros((KA * P, CH), np.float16)
    W1a[:CIN] = W1.astype(np.float16)
    W1a[CIN] = b1.astype(np.float16)
    W2a = np.zeros((KA * P, COUT), np.float16)
    W2a[:CH] = W2.astype(np.float16)
    W2a[CH] = b2.astype(np.float16)

    in_maps = []
    for l in range(NC):
        sl = slice(l * NL, (l + 1) * NL)
        Xa = np.zeros((KA * P, NL), np.float32)
        Xa[:CIN] = X[sl].T
        Xa[CIN] = 1.0
        xt_h = np.ascontiguousarray(
            Xa.reshape(KA, P, MT, P).transpose(2, 1, 0, 3)
        ).reshape(MT, P, KA * P).astype(np.float16)
        a_h = np.ascontiguousarray(
            A[sl].reshape(MT, P, ET, P).transpose(2, 1, 0, 3)
        ).reshape(ET, P, MT * P).astype(np.float16)
        b_h = np.ascontiguousarray(Bm[:, sl]).astype(np.float16)
        in_maps.append(
            {"xt": xt_h, "w1": W1a, "w2": W2a, "a_t": a_h, "b_t": b_h}
        )
    return in_maps


last_result = None  # BassKernelResults of the most recent run (for test harness)


def kernel(feature_hyg, pair_v, pair_e, num_edges, W1, b1, W2, b2):
    global last_result
    assert int(num_edges) == E, f"kernel hardcodes E={E}, got {int(num_edges)}"
    in_maps = _prepare_inputs(feature_hyg, pair_v, pair_e, W1, b1, W2, b2)
    if "nc" not in _CACHE:
        _CACHE["nc"] = _build()
    res = run_bass_kernel_spmd(_CACHE["nc"], in_maps, core_ids=list(range(NC)))
    last_result = res
    out = np.concatenate([res.results[l]["out"] for l in range(NC)], axis=0)
    return np.ascontiguousarray(out.astype(np.float32))


# revision 10
# speedup vs baseline: 1.0127x; 1.0127x over previous
"""HGNN+ (2x HGNNPConv) Trainium2 kernel, 8-core SPMD.

Strategy: the hypergraph v2v mean aggregation is a linear operator
    v2v(X) = Dv^-1 H De^-1 H^T X
with H the [N, E] incidence-count matrix. We materialize H on the host
(pure index preprocessing of pair_v/pair_e), fold the 1/De and 1/Dv
normalizations into column-scaled variants A = H/De (per edge) and
B = (H/Dv)^T (per vertex), shard vertices across the 8 cores, and run
the whole network as a chain of dense matmuls on the TensorEngine:

  per core l (NL = N/8 = 2048 local vertices):
    M1  = relu(Xl @ W1 + b1)            [NL, CH]    (fp16 in, fp32 psum)
    E1p = A_l^T @ M1                    [E, CH]     partial over local vertices
    E1  = AllReduce(E1p)                [E, CH]     (fp16)
    V1t = relu(E1^T-contract B_l)       [CH, NL]    = relu((B_l^T E1)^T)
    M2  = relu(V1^T @ W2 + b2)          [NL, COUT]
    E2p = A_l^T @ M2 ; E2 = AllReduce   [E, COUT]
    OUT = B_l^T @ E2                    [NL, COUT]  (fp32 out)

Layer 1 is pipelined over the two 512-wide channel halves: steps 1+2
for half c0, AllReduce(c0) in flight while steps 1+2 run for c1, and
step 3's first output rows (which only read E1[:, c0]) overlap
AllReduce(c1). Layer 2 chunks its AllReduce over edge-row halves.

Biases are folded in exactly via an augmented contraction row (row 1024
of the padded weight matrices is the bias; the matching activation row
is constant 1), so the kernel is correct for any b1/b2.

All matmul operands are fp16 (PE runs fp16 at full rate; psum
accumulation is fp32), which keeps the error ~5e-4 vs the fp32
reference. Layout choices keep every DMA's innermost run >= 1KB and
avoid all on-device transposes. Bulk streams (A, B) ride the ACT HWDGE
ring; latency-critical loads ride the Sync ring.
"""

import numpy as np

import concourse.bass as bass  # noqa: F401  (bass types used via bacc)
import concourse.mybir as mybir
import concourse.tile as tile
from concourse import bacc
from concourse.bass_utils import run_bass_kernel_spmd

# Problem shapes (hardcoded per spec nn_HGNNP_33629593927812)
N, E, CIN, CH, COUT = 16384, 2048, 1024, 1024, 512
NC = 8                # cores
NL = N // NC          # 2048 local vertices per core
P = 128
KA = CIN // P + 1     # 9 contraction tiles for the augmented (bias) matmuls
MT = NL // P          # 16 local-vertex tiles
ET = E // P           # 16 edge tiles
CHT = CH // 512       # 2 channel halves of the hidden dim
ARC = 2               # layer-2 AllReduce chunks (over edge rows)
EPC = ET // ARC       # edge tiles per layer-2 AllReduce chunk

F16 = mybir.dt.float16
F32 = mybir.dt.float32
RELU = mybir.ActivationFunctionType.Relu

_CACHE: dict = {}


def _build(with_bias: bool):
    """Build the per-core Bass program (identical on all 8 cores).

    with_bias=False drops the augmented bias contraction tile (the spec
    always feeds zero biases; the False variant saves ~48 matmuls).
    """
    nc = bacc.Bacc(None, target_bir_lowering=False, num_devices=NC)
    ka = KA if with_bias else CIN // P

    # Per-core inputs (host-prepared layouts; see kernel() below)
    xt = nc.dram_tensor("xt", [MT, P, ka * P], F16, kind="ExternalInput")
    w1 = nc.dram_tensor("w1", [ka * P, CH], F16, kind="ExternalInput")
    w2 = nc.dram_tensor("w2", [ka * P, COUT], F16, kind="ExternalInput")
    a_t = nc.dram_tensor("a_t", [ET, P, NL], F16, kind="ExternalInput")
    b_t = nc.dram_tensor("b_t", [E, NL], F16, kind="ExternalInput")
    out = nc.dram_tensor("out", [NL, COUT], F32, kind="ExternalOutput")

    RG = [list(range(NC))]
    # layer-2 AllReduce: asymmetric edge-row chunks — the small tail chunk
    # minimizes the exposed latency between step 5 and step 6
    L2C = [(0, 12), (12, 4)]

    with tile.TileContext(nc) as tc:
        with (
            tc.tile_pool(name="persist", bufs=1) as persist,
            tc.tile_pool(name="stream", bufs=3) as stream,
            tc.tile_pool(name="stage", bufs=6) as stage,
            tc.tile_pool(name="psum", bufs=6, space="PSUM") as psum_pool,
            tc.tile_pool(name="dram", bufs=1, space="DRAM") as dram,
        ):
            # ---- resident weights (sync ring: w1 needed immediately) ----
            w1_sb = persist.tile([P, ka, CH], F16, tag="slot_w")
            nc.sync.dma_start(w1_sb[:], w1.rearrange("(k pi) c -> pi k c", pi=P))
            w2_sb = persist.tile([P, ka, COUT], F16)

            # B_l resident; chunk loads are interleaved into step 2's stream
            # (B is only needed at step 3 — loading it up front starves the
            # startup-critical w1/xt transfers of SDMA bandwidth)
            b_sb = persist.tile([P, ET, NL], F16)  # 8MB
            b_v = b_t.rearrange("(po pi) v -> pi po v", pi=P)

            # AllReduce bounce buffers (collectives need internal DRAM tiles;
            # Shared output tiles are single-writer => one tile per chunk)
            e1p_d = [
                dram.tile([E, 512], F16, name=f"e1p_{n}") for n in range(CHT)
            ]
            e1r_d = [
                dram.tile([E, 512], F16, addr_space="Shared", name=f"e1r_{n}")
                for n in range(CHT)
            ]
            e2p_d = [
                dram.tile([nt * P, COUT], F16, name=f"e2p_{h}")
                for h, (_, nt) in enumerate(L2C)
            ]
            e2r_d = [
                dram.tile([nt * P, COUT], F16, addr_space="Shared", name=f"e2r_{h}")
                for h, (_, nt) in enumerate(L2C)
            ]
            e1p_v = [t.rearrange("(po pi) c -> pi po c", pi=P) for t in e1p_d]
            e2p_v = [t.rearrange("(po pi) c -> pi po c", pi=P) for t in e2p_d]

            m1_sb = persist.tile([P, MT, CH], F16, tag="slot_a")  # 4MB
            e1_sb = persist.tile([P, ET, CH], F16, tag="slot_e")  # 4MB

            # ---- layer 1, pipelined over channel halves ----
            for n in range(CHT):
                cs = slice(n * 512, (n + 1) * 512)
                # step 1: M1[:, cs] = relu(X @ W1 + b1)[:, cs]
                for m in range(MT):
                    xt_sb = stream.tile([P, ka * P], F16, tag="stream")
                    nc.sync.dma_start(xt_sb[:], xt[m])
                    ps = psum_pool.tile([P, 512], F32, tag="ps")
                    for k in range(ka):
                        nc.tensor.matmul(
                            ps[:],
                            xt_sb[:, k * P:(k + 1) * P],
                            w1_sb[:, k, cs],
                            start=(k == 0),
                            stop=(k == ka - 1),
                        )
                    nc.scalar.activation(m1_sb[:, m, cs], ps[:], RELU)
                # step 2: E1p[:, cs] = A^T M1[:, cs]
                for me in range(ET):
                    a_sb = stream.tile([P, NL], F16, tag="stream")
                    nc.scalar.dma_start(a_sb[:], a_t[me])
                    if n == 0:
                        # sneak one B chunk load in behind each A tile
                        nc.scalar.dma_start(b_sb[:, me, :], b_v[:, me, :])
                    ps = psum_pool.tile([P, 512], F32, tag="ps")
                    for k in range(MT):
                        nc.tensor.matmul(
                            ps[:],
                            a_sb[:, k * P:(k + 1) * P],
                            m1_sb[:, k, cs],
                            start=(k == 0),
                            stop=(k == MT - 1),
                        )
                    st = stage.tile([P, 512], F16, tag="stage")
                    nc.vector.tensor_copy(st[:], ps[:])
                    nc.sync.dma_start(e1p_v[n][:, me, :], st[:])
                nc.gpsimd.collective_compute(
                    "AllReduce",
                    mybir.AluOpType.add,
                    replica_groups=RG,
                    ins=[e1p_d[n].opt()],
                    outs=[e1r_d[n].opt()],
                )

            # E1 back to SBUF as lhsT tiles [e_pi, e_po, c]. These loads wait
            # on the AllReduces, so they are issued only after both collective
            # triggers (a waiting DMA head-of-line-blocks its HWDGE ring).
            for n in range(CHT):
                nc.sync.dma_start(
                    e1_sb[:, :, n * 512:(n + 1) * 512],
                    e1r_d[n].rearrange("(po pi) c -> pi po c", pi=P),
                )
            nc.sync.dma_start(w2_sb[:], w2.rearrange("(k pi) c -> pi k c", pi=P))

            # ---- step 3: V1t = relu(sum_e E1[e,c] B[e,v]), [ch, vl] ----
            # output row-block mc only reads E1[:, mc*128:...] => the first
            # half overlaps AllReduce(c1).
            # ka slots: CH//P data tiles, plus (with bias) the constant row
            v1t_sb = persist.tile([P, ka, NL], F16, tag="slot_a")
            if with_bias:
                nc.vector.memset(v1t_sb[:, ka - 1, :], 0.0)
                nc.vector.memset(v1t_sb[0:1, ka - 1, :], 1.0)  # bias row (ch==CH)
            for mc in range(CH // P):
                for nv in range(NL // 512):
                    ps = psum_pool.tile([P, 512], F32, tag="ps")
                    for k in range(ET):
                        nc.tensor.matmul(
                            ps[:],
                            e1_sb[:, k, mc * P:(mc + 1) * P],
                            b_sb[:, k, nv * 512:(nv + 1) * 512],
                            start=(k == 0),
                            stop=(k == ET - 1),
                        )
                    nc.scalar.activation(
                        v1t_sb[:, mc, nv * 512:(nv + 1) * 512], ps[:], RELU
                    )

            # ---- step 4: M2 = relu(V1 @ W2 + b2), [vl, c2] ----
            m2_sb = persist.tile([P, MT, COUT], F16, tag="slot_w")  # 2MB
            for m in range(MT):
                ps = psum_pool.tile([P, 512], F32, tag="ps")
                for k in range(ka):
                    nc.tensor.matmul(
                        ps[:],
                        v1t_sb[:, k, m * P:(m + 1) * P],
                        w2_sb[:, k, :],
                        start=(k == 0),
                        stop=(k == ka - 1),
                    )
                nc.scalar.activation(m2_sb[:, m, :], ps[:], RELU)

            # ---- step 5: E2p = A^T M2; asymmetric edge-row AllReduce ----
            for h, (t0, nt) in enumerate(L2C):
                for me in range(t0, t0 + nt):
                    a_sb = stream.tile([P, NL], F16, tag="stream")
                    nc.scalar.dma_start(a_sb[:], a_t[me])
                    ps = psum_pool.tile([P, 512], F32, tag="ps")
                    for k in range(MT):
                        nc.tensor.matmul(
                            ps[:],
                            a_sb[:, k * P:(k + 1) * P],
                            m2_sb[:, k, :],
                            start=(k == 0),
                            stop=(k == MT - 1),
                        )
                    st = stage.tile([P, 512], F16, tag="stage")
                    nc.vector.tensor_copy(st[:], ps[:])
                    nc.sync.dma_start(e2p_v[h][:, me - t0, :], st[:])
                nc.gpsimd.collective_compute(
                    "AllReduce",
                    mybir.AluOpType.add,
                    replica_groups=RG,
                    ins=[e2p_d[h].opt()],
                    outs=[e2r_d[h].opt()],
                )

            e2_sb = persist.tile([P, ET, COUT], F16, tag="slot_e")  # 2MB
            for h, (t0, nt) in enumerate(L2C):
                nc.sync.dma_start(
                    e2_sb[:, t0:t0 + nt, :],
                    e2r_d[h].rearrange("(po pi) c -> pi po c", pi=P),
                )

            # ---- step 6: OUT = sum_e B[e,v] E2[e,c2], [vl, c2] ----
            out_v = out.rearrange("(po pi) c -> pi po c", pi=P)
            for m in range(MT):
                ps = psum_pool.tile([P, 512], F32, tag="ps")
                for k in range(ET):
                    nc.tensor.matmul(
                        ps[:],
                        b_sb[:, k, m * P:(m + 1) * P],
                        e2_sb[:, k, :],
                        start=(k == 0),
                        stop=(k == ET - 1),
                    )
                st = stage.tile([P, 512], F32, tag="stage_out")
                nc.vector.tensor_copy(st[:], ps[:])
                nc.sync.dma_start(out_v[:, m, :], st[:])

    nc.compile()
    return nc


def _prepare_inputs(feature_hyg, pair_v, pair_e, W1, b1, W2, b2, with_bias):
    X = np.ascontiguousarray(np.asarray(feature_hyg, dtype=np.float32))
    pv = np.asarray(pair_v).astype(np.int64)
    pe = np.asarray(pair_e).astype(np.int64)
    W1 = np.asarray(W1, dtype=np.float32)
    b1 = np.asarray(b1, dtype=np.float32)
    W2 = np.asarray(W2, dtype=np.float32)
    b2 = np.asarray(b2, dtype=np.float32)
    ka = KA if with_bias else CIN // P

    ec = np.bincount(pe, minlength=E).astype(np.float32)
    vc = np.bincount(pv, minlength=N).astype(np.float32)
    H = (
        np.bincount(pv * E + pe, minlength=N * E)
        .astype(np.float32)
        .reshape(N, E)
    )
    A = H / np.maximum(ec, 1.0)[None, :]          # [N, E], col-scaled by 1/De
    Bm = (H / np.maximum(vc, 1.0)[:, None]).T     # [E, N], col-scaled by 1/Dv

    W1a = np.zeros((ka * P, CH), np.float16)
    W1a[:CIN] = W1.astype(np.float16)
    W2a = np.zeros((ka * P, COUT), np.float16)
    W2a[:CH] = W2.astype(np.float16)
    if with_bias:
        W1a[CIN] = b1.astype(np.float16)
        W2a[CH] = b2.astype(np.float16)

    in_maps = []
    for l in range(NC):
        sl = slice(l * NL, (l + 1) * NL)
        Xa = np.zeros((ka * P, NL), np.float32)
        Xa[:CIN] = X[sl].T
        if with_bias:
            Xa[CIN] = 1.0
        xt_h = np.ascontiguousarray(
            Xa.reshape(ka, P, MT, P).transpose(2, 1, 0, 3)
        ).reshape(MT, P, ka * P).astype(np.float16)
        a_h = np.ascontiguousarray(
            A[sl].reshape(MT, P, ET, P).transpose(2, 1, 0, 3)
        ).reshape(ET, P, MT * P).astype(np.float16)
        b_h = np.ascontiguousarray(Bm[:, sl]).astype(np.float16)
        in_maps.append(
            {"xt": xt_h, "w1": W1a, "w2": W2a, "a_t": a_h, "b_t": b_h}
        )
    return in_maps


last_result = None  # BassKernelResults of the most recent run (for test harness)


def kernel(feature_hyg, pair_v, pair_e, num_edges, W1, b1, W2, b2):
    global last_result
    assert int(num_edges) == E, f"kernel hardcodes E={E}, got {int(num_edges)}"
    with_bias = bool(np.any(np.asarray(b1)) or np.any(np.asarray(b2)))
    in_maps = _prepare_inputs(
        feature_hyg, pair_v, pair_e, W1, b1, W2, b2, with_bias
    )
    key = ("nc", with_bias)
    if key not in _CACHE:
        _CACHE[key] = _build(with_bias)
    res = run_bass_kernel_spmd(_CACHE[key], in_maps, core_ids=list(range(NC)))
    last_result = res
    out = np.concatenate([res.results[l]["out"] for l in range(NC)], axis=0)
    return np.ascontiguousarray(out.astype(np.float32))


# revision 11
# speedup vs baseline: 1.0399x; 1.0268x over previous
"""HGNN+ (2x HGNNPConv) Trainium2 kernel, 8-core SPMD.

Strategy: the hypergraph v2v mean aggregation is a linear operator
    v2v(X) = Dv^-1 H De^-1 H^T X
with H the [N, E] incidence-count matrix. We materialize H on the host
(pure index preprocessing of pair_v/pair_e), fold the 1/De and 1/Dv
normalizations into column-scaled variants A = H/De (per edge) and
B = (H/Dv)^T (per vertex), shard vertices across the 8 cores, and run
the whole network as a chain of dense matmuls on the TensorEngine:

  per core l (NL = N/8 = 2048 local vertices):
    M1  = relu(Xl @ W1 + b1)            [NL, CH]    (fp16 in, fp32 psum)
    E1p = A_l^T @ M1                    [E, CH]     partial over local vertices
    E1  = AllReduce(E1p)                [E, CH]     (fp16)
    V1t = relu(E1^T-contract B_l)       [CH, NL]    = relu((B_l^T E1)^T)
    M2  = relu(V1^T @ W2 + b2)          [NL, COUT]
    E2p = A_l^T @ M2 ; E2 = AllReduce   [E, COUT]
    OUT = B_l^T @ E2                    [NL, COUT]  (fp32 out)

Layer 1 is pipelined over the two 512-wide channel halves: steps 1+2
for half c0, AllReduce(c0) in flight while steps 1+2 run for c1, and
step 3's first output rows (which only read E1[:, c0]) overlap
AllReduce(c1). Layer 2 chunks its AllReduce over edge-row halves.

Biases are folded in exactly via an augmented contraction row (row 1024
of the padded weight matrices is the bias; the matching activation row
is constant 1), so the kernel is correct for any b1/b2.

All matmul operands are fp16 (PE runs fp16 at full rate; psum
accumulation is fp32), which keeps the error ~5e-4 vs the fp32
reference. Layout choices keep every DMA's innermost run >= 1KB and
avoid all on-device transposes. Bulk streams (A, B) ride the ACT HWDGE
ring; latency-critical loads ride the Sync ring.
"""

import numpy as np

import concourse.bass as bass  # noqa: F401  (bass types used via bacc)
import concourse.mybir as mybir
import concourse.tile as tile
from concourse import bacc
from concourse.bass_utils import run_bass_kernel_spmd

# Problem shapes (hardcoded per spec nn_HGNNP_33629593927812)
N, E, CIN, CH, COUT = 16384, 2048, 1024, 1024, 512
NC = 8                # cores
NL = N // NC          # 2048 local vertices per core
P = 128
KA = CIN // P + 1     # 9 contraction tiles for the augmented (bias) matmuls
MT = NL // P          # 16 local-vertex tiles
ET = E // P           # 16 edge tiles
CHT = CH // 512       # 2 channel halves of the hidden dim
ARC = 2               # layer-2 AllReduce chunks (over edge rows)
EPC = ET // ARC       # edge tiles per layer-2 AllReduce chunk

F16 = mybir.dt.float16
F32 = mybir.dt.float32
RELU = mybir.ActivationFunctionType.Relu

_CACHE: dict = {}


def _build(with_bias: bool):
    """Build the per-core Bass program (identical on all 8 cores).

    with_bias=False drops the augmented bias contraction tile (the spec
    always feeds zero biases; the False variant saves ~48 matmuls).
    """
    nc = bacc.Bacc(None, target_bir_lowering=False, num_devices=NC)
    ka = KA if with_bias else CIN // P

    # Per-core inputs (host-prepared layouts; see kernel() below)
    xt = nc.dram_tensor("xt", [MT, P, ka * P], F16, kind="ExternalInput")
    w1 = nc.dram_tensor("w1", [ka * P, CH], F16, kind="ExternalInput")
    w2 = nc.dram_tensor("w2", [ka * P, COUT], F16, kind="ExternalInput")
    a_t = nc.dram_tensor("a_t", [ET, P, NL], F16, kind="ExternalInput")
    b_t = nc.dram_tensor("b_t", [E, NL], F16, kind="ExternalInput")
    out = nc.dram_tensor("out", [NL, COUT], F32, kind="ExternalOutput")

    RG = [list(range(NC))]
    # layer-2 AllReduce: symmetric edge-row chunks (measured best: the
    # first chunk's AllReduce hides under step 5's second half, the second
    # chunk's under step 6's first psum groups)
    L2C = [(0, 8), (8, 8)]

    with tile.TileContext(nc) as tc:
        with (
            tc.tile_pool(name="persist", bufs=1) as persist,
            tc.tile_pool(name="stream", bufs=3) as stream,
            tc.tile_pool(name="stage", bufs=6) as stage,
            tc.tile_pool(name="psum", bufs=6, space="PSUM") as psum_pool,
            tc.tile_pool(name="dram", bufs=1, space="DRAM") as dram,
        ):
            # ---- resident weights (sync ring: w1 needed immediately) ----
            w1_sb = persist.tile([P, ka, CH], F16, tag="slot_w")
            w1_v = w1.rearrange("(k pi) c -> pi k c", pi=P)
            # split by channel half: step 1's first psum group only needs
            # the c0 half, so the first matmul starts ~2x sooner
            nc.sync.dma_start(w1_sb[:, :, 0:512], w1_v[:, :, 0:512])
            nc.sync.dma_start(w1_sb[:, :, 512:1024], w1_v[:, :, 512:1024])
            w2_sb = persist.tile([P, ka, COUT], F16)

            # B_l resident; chunk loads are interleaved into step 2's stream
            # (B is only needed at step 3 — loading it up front starves the
            # startup-critical w1/xt transfers of SDMA bandwidth)
            b_sb = persist.tile([P, ET, NL], F16)  # 8MB
            b_v = b_t.rearrange("(po pi) v -> pi po v", pi=P)

            # AllReduce bounce buffers (collectives need internal DRAM tiles;
            # Shared output tiles are single-writer => one tile per chunk)
            e1p_d = [
                dram.tile([E, 512], F16, name=f"e1p_{n}") for n in range(CHT)
            ]
            e1r_d = [
                dram.tile([E, 512], F16, addr_space="Shared", name=f"e1r_{n}")
                for n in range(CHT)
            ]
            e2p_d = [
                dram.tile([nt * P, COUT], F16, name=f"e2p_{h}")
                for h, (_, nt) in enumerate(L2C)
            ]
            e2r_d = [
                dram.tile([nt * P, COUT], F16, addr_space="Shared", name=f"e2r_{h}")
                for h, (_, nt) in enumerate(L2C)
            ]
            e1p_v = [t.rearrange("(po pi) c -> pi po c", pi=P) for t in e1p_d]
            e2p_v = [t.rearrange("(po pi) c -> pi po c", pi=P) for t in e2p_d]

            m1_sb = persist.tile([P, MT, CH], F16, tag="slot_a")  # 4MB
            e1_sb = persist.tile([P, ET, CH], F16, tag="slot_e")  # 4MB

            # ---- layer 1, pipelined over channel halves ----
            for n in range(CHT):
                cs = slice(n * 512, (n + 1) * 512)
                # step 1: M1[:, cs] = relu(X @ W1 + b1)[:, cs]
                for m in range(MT):
                    xt_sb = stream.tile([P, ka * P], F16, tag="stream")
                    nc.sync.dma_start(xt_sb[:], xt[m])
                    ps = psum_pool.tile([P, 512], F32, tag="ps")
                    for k in range(ka):
                        nc.tensor.matmul(
                            ps[:],
                            xt_sb[:, k * P:(k + 1) * P],
                            w1_sb[:, k, cs],
                            start=(k == 0),
                            stop=(k == ka - 1),
                        )
                    nc.scalar.activation(m1_sb[:, m, cs], ps[:], RELU)
                # step 2: E1p[:, cs] = A^T M1[:, cs]
                for me in range(ET):
                    a_sb = stream.tile([P, NL], F16, tag="stream")
                    nc.scalar.dma_start(a_sb[:], a_t[me])
                    if n == 0:
                        # sneak one B chunk load in behind each A tile
                        nc.scalar.dma_start(b_sb[:, me, :], b_v[:, me, :])
                    ps = psum_pool.tile([P, 512], F32, tag="ps")
                    for k in range(MT):
                        nc.tensor.matmul(
                            ps[:],
                            a_sb[:, k * P:(k + 1) * P],
                            m1_sb[:, k, cs],
                            start=(k == 0),
                            stop=(k == MT - 1),
                        )
                    st = stage.tile([P, 512], F16, tag="stage")
                    nc.vector.tensor_copy(st[:], ps[:])
                    nc.sync.dma_start(e1p_v[n][:, me, :], st[:])
                nc.gpsimd.collective_compute(
                    "AllReduce",
                    mybir.AluOpType.add,
                    replica_groups=RG,
                    ins=[e1p_d[n].opt()],
                    outs=[e1r_d[n].opt()],
                )

            # E1 back to SBUF as lhsT tiles [e_pi, e_po, c]. These loads wait
            # on the AllReduces, so they are issued only after both collective
            # triggers (a waiting DMA head-of-line-blocks its HWDGE ring).
            for n in range(CHT):
                nc.sync.dma_start(
                    e1_sb[:, :, n * 512:(n + 1) * 512],
                    e1r_d[n].rearrange("(po pi) c -> pi po c", pi=P),
                )
            nc.sync.dma_start(w2_sb[:], w2.rearrange("(k pi) c -> pi k c", pi=P))

            # ---- step 3: V1t = relu(sum_e E1[e,c] B[e,v]), [ch, vl] ----
            # output row-block mc only reads E1[:, mc*128:...] => the first
            # half overlaps AllReduce(c1).
            # ka slots: CH//P data tiles, plus (with bias) the constant row
            v1t_sb = persist.tile([P, ka, NL], F16, tag="slot_a")
            if with_bias:
                nc.vector.memset(v1t_sb[:, ka - 1, :], 0.0)
                nc.vector.memset(v1t_sb[0:1, ka - 1, :], 1.0)  # bias row (ch==CH)
            for mc in range(CH // P):
                for nv in range(NL // 512):
                    ps = psum_pool.tile([P, 512], F32, tag="ps")
                    for k in range(ET):
                        nc.tensor.matmul(
                            ps[:],
                            e1_sb[:, k, mc * P:(mc + 1) * P],
                            b_sb[:, k, nv * 512:(nv + 1) * 512],
                            start=(k == 0),
                            stop=(k == ET - 1),
                        )
                    nc.scalar.activation(
                        v1t_sb[:, mc, nv * 512:(nv + 1) * 512], ps[:], RELU
                    )

            # ---- step 4: M2 = relu(V1 @ W2 + b2), [vl, c2] ----
            m2_sb = persist.tile([P, MT, COUT], F16, tag="slot_w")  # 2MB
            for m in range(MT):
                ps = psum_pool.tile([P, 512], F32, tag="ps")
                for k in range(ka):
                    nc.tensor.matmul(
                        ps[:],
                        v1t_sb[:, k, m * P:(m + 1) * P],
                        w2_sb[:, k, :],
                        start=(k == 0),
                        stop=(k == ka - 1),
                    )
                nc.scalar.activation(m2_sb[:, m, :], ps[:], RELU)

            # ---- step 5: E2p = A^T M2; asymmetric edge-row AllReduce ----
            for h, (t0, nt) in enumerate(L2C):
                for me in range(t0, t0 + nt):
                    a_sb = stream.tile([P, NL], F16, tag="stream")
                    nc.scalar.dma_start(a_sb[:], a_t[me])
                    ps = psum_pool.tile([P, 512], F32, tag="ps")
                    for k in range(MT):
                        nc.tensor.matmul(
                            ps[:],
                            a_sb[:, k * P:(k + 1) * P],
                            m2_sb[:, k, :],
                            start=(k == 0),
                            stop=(k == MT - 1),
                        )
                    st = stage.tile([P, 512], F16, tag="stage")
                    nc.vector.tensor_copy(st[:], ps[:])
                    nc.sync.dma_start(e2p_v[h][:, me - t0, :], st[:])
                nc.gpsimd.collective_compute(
                    "AllReduce",
                    mybir.AluOpType.add,
                    replica_groups=RG,
                    ins=[e2p_d[h].opt()],
                    outs=[e2r_d[h].opt()],
                )

            e2_sb = persist.tile([P, ET, COUT], F16, tag="slot_e")  # 2MB
            for h, (t0, nt) in enumerate(L2C):
                nc.sync.dma_start(
                    e2_sb[:, t0:t0 + nt, :],
                    e2r_d[h].rearrange("(po pi) c -> pi po c", pi=P),
                )

            # ---- step 6: OUT = sum_e B[e,v] E2[e,c2], [vl, c2] ----
            out_v = out.rearrange("(po pi) c -> pi po c", pi=P)
            for m in range(MT):
                ps = psum_pool.tile([P, 512], F32, tag="ps")
                for k in range(ET):
                    nc.tensor.matmul(
                        ps[:],
                        b_sb[:, k, m * P:(m + 1) * P],
                        e2_sb[:, k, :],
                        start=(k == 0),
                        stop=(k == ET - 1),
                    )
                st = stage.tile([P, 512], F32, tag="stage_out")
                nc.vector.tensor_copy(st[:], ps[:])
                nc.sync.dma_start(out_v[:, m, :], st[:])

    nc.compile()
    return nc


def _prepare_inputs(feature_hyg, pair_v, pair_e, W1, b1, W2, b2, with_bias):
    X = np.ascontiguousarray(np.asarray(feature_hyg, dtype=np.float32))
    pv = np.asarray(pair_v).astype(np.int64)
    pe = np.asarray(pair_e).astype(np.int64)
    W1 = np.asarray(W1, dtype=np.float32)
    b1 = np.asarray(b1, dtype=np.float32)
    W2 = np.asarray(W2, dtype=np.float32)
    b2 = np.asarray(b2, dtype=np.float32)
    ka = KA if with_bias else CIN // P

    ec = np.bincount(pe, minlength=E).astype(np.float32)
    vc = np.bincount(pv, minlength=N).astype(np.float32)
    H = (
        np.bincount(pv * E + pe, minlength=N * E)
        .astype(np.float32)
        .reshape(N, E)
    )
    A = H / np.maximum(ec, 1.0)[None, :]          # [N, E], col-scaled by 1/De
    Bm = (H / np.maximum(vc, 1.0)[:, None]).T     # [E, N], col-scaled by 1/Dv

    W1a = np.zeros((ka * P, CH), np.float16)
    W1a[:CIN] = W1.astype(np.float16)
    W2a = np.zeros((ka * P, COUT), np.float16)
    W2a[:CH] = W2.astype(np.float16)
    if with_bias:
        W1a[CIN] = b1.astype(np.float16)
        W2a[CH] = b2.astype(np.float16)

    in_maps = []
    for l in range(NC):
        sl = slice(l * NL, (l + 1) * NL)
        Xa = np.zeros((ka * P, NL), np.float32)
        Xa[:CIN] = X[sl].T
        if with_bias:
            Xa[CIN] = 1.0
        xt_h = np.ascontiguousarray(
            Xa.reshape(ka, P, MT, P).transpose(2, 1, 0, 3)
        ).reshape(MT, P, ka * P).astype(np.float16)
        a_h = np.ascontiguousarray(
            A[sl].reshape(MT, P, ET, P).transpose(2, 1, 0, 3)
        ).reshape(ET, P, MT * P).astype(np.float16)
        b_h = np.ascontiguousarray(Bm[:, sl]).astype(np.float16)
        in_maps.append(
            {"xt": xt_h, "w1": W1a, "w2": W2a, "a_t": a_h, "b_t": b_h}
        )
    return in_maps


last_result = None  # BassKernelResults of the most recent run (for test harness)


def kernel(feature_hyg, pair_v, pair_e, num_edges, W1, b1, W2, b2):
    global last_result
    assert int(num_edges) == E, f"kernel hardcodes E={E}, got {int(num_edges)}"
    with_bias = bool(np.any(np.asarray(b1)) or np.any(np.asarray(b2)))
    in_maps = _prepare_inputs(
        feature_hyg, pair_v, pair_e, W1, b1, W2, b2, with_bias
    )
    key = ("nc", with_bias)
    if key not in _CACHE:
        _CACHE[key] = _build(with_bias)
    res = run_bass_kernel_spmd(_CACHE[key], in_maps, core_ids=list(range(NC)))
    last_result = res
    out = np.concatenate([res.results[l]["out"] for l in range(NC)], axis=0)
    return np.ascontiguousarray(out.astype(np.float32))


# revision 12
# speedup vs baseline: 1.0559x; 1.0154x over previous
"""HGNN+ (2x HGNNPConv) Trainium2 kernel, 8-core SPMD.

Strategy: the hypergraph v2v mean aggregation is a linear operator
    v2v(X) = Dv^-1 H De^-1 H^T X
with H the [N, E] incidence-count matrix. We materialize H on the host
(pure index preprocessing of pair_v/pair_e), fold the 1/De and 1/Dv
normalizations into column-scaled variants A = H/De (per edge) and
B = (H/Dv)^T (per vertex), shard vertices across the 8 cores, and run
the whole network as a chain of dense matmuls on the TensorEngine:

  per core l (NL = N/8 = 2048 local vertices):
    M1  = relu(Xl @ W1 + b1)            [NL, CH]    (fp16 in, fp32 psum)
    E1p = A_l^T @ M1                    [E, CH]     partial over local vertices
    E1  = AllReduce(E1p)                [E, CH]     (fp16)
    V1t = relu(E1^T-contract B_l)       [CH, NL]    = relu((B_l^T E1)^T)
    M2  = relu(V1^T @ W2 + b2)          [NL, COUT]
    E2p = A_l^T @ M2 ; E2 = AllReduce   [E, COUT]
    OUT = B_l^T @ E2                    [NL, COUT]  (fp32 out)

Layer 1 is pipelined over the two 512-wide channel halves: steps 1+2
for half c0, AllReduce(c0) in flight while steps 1+2 run for c1, and
step 3's first output rows (which only read E1[:, c0]) overlap
AllReduce(c1). Layer 2 chunks its AllReduce over edge-row halves.

Biases are folded in exactly via an augmented contraction row (row 1024
of the padded weight matrices is the bias; the matching activation row
is constant 1), so the kernel is correct for any b1/b2.

All matmul operands are fp16 (PE runs fp16 at full rate; psum
accumulation is fp32), which keeps the error ~5e-4 vs the fp32
reference. Layout choices keep every DMA's innermost run >= 1KB and
avoid all on-device transposes. Bulk streams (A, B) ride the ACT HWDGE
ring; latency-critical loads ride the Sync ring.
"""

import numpy as np

import concourse.bass as bass  # noqa: F401  (bass types used via bacc)
import concourse.mybir as mybir
import concourse.tile as tile
from concourse import bacc
from concourse.bass_utils import run_bass_kernel_spmd

# Problem shapes (hardcoded per spec nn_HGNNP_33629593927812)
N, E, CIN, CH, COUT = 16384, 2048, 1024, 1024, 512
NC = 8                # cores
NL = N // NC          # 2048 local vertices per core
P = 128
KA = CIN // P + 1     # 9 contraction tiles for the augmented (bias) matmuls
MT = NL // P          # 16 local-vertex tiles
ET = E // P           # 16 edge tiles
CHT = CH // 512       # 2 channel halves of the hidden dim
ARC = 2               # layer-2 AllReduce chunks (over edge rows)
EPC = ET // ARC       # edge tiles per layer-2 AllReduce chunk

F16 = mybir.dt.float16
F32 = mybir.dt.float32
RELU = mybir.ActivationFunctionType.Relu

_CACHE: dict = {}


def _build(with_bias: bool):
    """Build the per-core Bass program (identical on all 8 cores).

    with_bias=False drops the augmented bias contraction tile (the spec
    always feeds zero biases; the False variant saves ~48 matmuls).
    """
    nc = bacc.Bacc(None, target_bir_lowering=False, num_devices=NC)
    ka = KA if with_bias else CIN // P

    # Per-core inputs (host-prepared layouts; see kernel() below)
    xt = nc.dram_tensor("xt", [MT, P, ka * P], F16, kind="ExternalInput")
    w1 = nc.dram_tensor("w1", [ka * P, CH], F16, kind="ExternalInput")
    w2 = nc.dram_tensor("w2", [ka * P, COUT], F16, kind="ExternalInput")
    a_t = nc.dram_tensor("a_t", [ET, P, NL], F16, kind="ExternalInput")
    b_t = nc.dram_tensor("b_t", [E, NL], F16, kind="ExternalInput")
    out = nc.dram_tensor("out", [NL, COUT], F32, kind="ExternalOutput")

    RG = [list(range(NC))]
    # layer-2 AllReduce: symmetric edge-row chunks (measured best: the
    # first chunk's AllReduce hides under step 5's second half, the second
    # chunk's under step 6's first psum groups)
    L2C = [(0, 8), (8, 8)]

    with tile.TileContext(nc) as tc:
        with (
            tc.tile_pool(name="persist", bufs=1) as persist,
            tc.tile_pool(name="stream", bufs=4) as stream,
            tc.tile_pool(name="stage", bufs=6) as stage,
            tc.tile_pool(name="psum", bufs=8, space="PSUM") as psum_pool,
            tc.tile_pool(name="dram", bufs=1, space="DRAM") as dram,
        ):
            # ---- resident weights (sync ring: w1 needed immediately) ----
            w1_sb = persist.tile([P, ka, CH], F16, tag="slot_w")
            w1_v = w1.rearrange("(k pi) c -> pi k c", pi=P)
            # split by channel half: step 1's first psum group only needs
            # the c0 half, so the first matmul starts ~2x sooner
            nc.sync.dma_start(w1_sb[:, :, 0:512], w1_v[:, :, 0:512])
            nc.sync.dma_start(w1_sb[:, :, 512:1024], w1_v[:, :, 512:1024])
            w2_sb = persist.tile([P, ka, COUT], F16)

            # B_l resident; chunk loads are interleaved into step 2's stream
            # (B is only needed at step 3 — loading it up front starves the
            # startup-critical w1/xt transfers of SDMA bandwidth)
            b_sb = persist.tile([P, ET, NL], F16)  # 8MB
            b_v = b_t.rearrange("(po pi) v -> pi po v", pi=P)

            # AllReduce bounce buffers (collectives need internal DRAM tiles;
            # Shared output tiles are single-writer => one tile per chunk)
            e1p_d = [
                dram.tile([E, 512], F16, name=f"e1p_{n}") for n in range(CHT)
            ]
            e1r_d = [
                dram.tile([E, 512], F16, addr_space="Shared", name=f"e1r_{n}")
                for n in range(CHT)
            ]
            e2p_d = [
                dram.tile([nt * P, COUT], F16, name=f"e2p_{h}")
                for h, (_, nt) in enumerate(L2C)
            ]
            e2r_d = [
                dram.tile([nt * P, COUT], F16, addr_space="Shared", name=f"e2r_{h}")
                for h, (_, nt) in enumerate(L2C)
            ]
            e1p_v = [t.rearrange("(po pi) c -> pi po c", pi=P) for t in e1p_d]
            e2p_v = [t.rearrange("(po pi) c -> pi po c", pi=P) for t in e2p_d]

            m1_sb = persist.tile([P, MT, CH], F16, tag="slot_a")  # 4MB
            e1_sb = persist.tile([P, ET, CH], F16, tag="slot_e")  # 4MB

            # ---- layer 1, pipelined over channel halves ----
            for n in range(CHT):
                cs = slice(n * 512, (n + 1) * 512)
                # step 1: M1[:, cs] = relu(X @ W1 + b1)[:, cs]
                for m in range(MT):
                    xt_sb = stream.tile([P, ka * P], F16, tag="stream")
                    nc.sync.dma_start(xt_sb[:], xt[m])
                    ps = psum_pool.tile([P, 512], F32, tag="ps")
                    for k in range(ka):
                        nc.tensor.matmul(
                            ps[:],
                            xt_sb[:, k * P:(k + 1) * P],
                            w1_sb[:, k, cs],
                            start=(k == 0),
                            stop=(k == ka - 1),
                        )
                    nc.scalar.activation(m1_sb[:, m, cs], ps[:], RELU)
                # step 2: E1p[:, cs] = A^T M1[:, cs]
                for me in range(ET):
                    a_sb = stream.tile([P, NL], F16, tag="stream")
                    nc.scalar.dma_start(a_sb[:], a_t[me])
                    if n == 0:
                        # sneak one B chunk load in behind each A tile
                        nc.scalar.dma_start(b_sb[:, me, :], b_v[:, me, :])
                    ps = psum_pool.tile([P, 512], F32, tag="ps")
                    for k in range(MT):
                        nc.tensor.matmul(
                            ps[:],
                            a_sb[:, k * P:(k + 1) * P],
                            m1_sb[:, k, cs],
                            start=(k == 0),
                            stop=(k == MT - 1),
                        )
                    st = stage.tile([P, 512], F16, tag="stage")
                    nc.vector.tensor_copy(st[:], ps[:])
                    nc.sync.dma_start(e1p_v[n][:, me, :], st[:])
                nc.gpsimd.collective_compute(
                    "AllReduce",
                    mybir.AluOpType.add,
                    replica_groups=RG,
                    ins=[e1p_d[n].opt()],
                    outs=[e1r_d[n].opt()],
                )

            # E1 back to SBUF as lhsT tiles [e_pi, e_po, c]. These loads wait
            # on the AllReduces, so they are issued only after both collective
            # triggers (a waiting DMA head-of-line-blocks its HWDGE ring).
            for n in range(CHT):
                nc.sync.dma_start(
                    e1_sb[:, :, n * 512:(n + 1) * 512],
                    e1r_d[n].rearrange("(po pi) c -> pi po c", pi=P),
                )
            nc.sync.dma_start(w2_sb[:], w2.rearrange("(k pi) c -> pi k c", pi=P))

            # ---- step 3: V1t = relu(sum_e E1[e,c] B[e,v]), [ch, vl] ----
            # output row-block mc only reads E1[:, mc*128:...] => the first
            # half overlaps AllReduce(c1).
            # ka slots: CH//P data tiles, plus (with bias) the constant row
            v1t_sb = persist.tile([P, ka, NL], F16, tag="slot_a")
            if with_bias:
                nc.vector.memset(v1t_sb[:, ka - 1, :], 0.0)
                nc.vector.memset(v1t_sb[0:1, ka - 1, :], 1.0)  # bias row (ch==CH)
            for mc in range(CH // P):
                for nv in range(NL // 512):
                    ps = psum_pool.tile([P, 512], F32, tag="ps")
                    for k in range(ET):
                        nc.tensor.matmul(
                            ps[:],
                            e1_sb[:, k, mc * P:(mc + 1) * P],
                            b_sb[:, k, nv * 512:(nv + 1) * 512],
                            start=(k == 0),
                            stop=(k == ET - 1),
                        )
                    nc.scalar.activation(
                        v1t_sb[:, mc, nv * 512:(nv + 1) * 512], ps[:], RELU
                    )

            # ---- step 4: M2 = relu(V1 @ W2 + b2), [vl, c2] ----
            m2_sb = persist.tile([P, MT, COUT], F16, tag="slot_w")  # 2MB
            for m in range(MT):
                ps = psum_pool.tile([P, 512], F32, tag="ps")
                for k in range(ka):
                    nc.tensor.matmul(
                        ps[:],
                        v1t_sb[:, k, m * P:(m + 1) * P],
                        w2_sb[:, k, :],
                        start=(k == 0),
                        stop=(k == ka - 1),
                    )
                nc.scalar.activation(m2_sb[:, m, :], ps[:], RELU)

            # ---- step 5: E2p = A^T M2; asymmetric edge-row AllReduce ----
            for h, (t0, nt) in enumerate(L2C):
                for me in range(t0, t0 + nt):
                    a_sb = stream.tile([P, NL], F16, tag="stream")
                    nc.scalar.dma_start(a_sb[:], a_t[me])
                    ps = psum_pool.tile([P, 512], F32, tag="ps")
                    for k in range(MT):
                        nc.tensor.matmul(
                            ps[:],
                            a_sb[:, k * P:(k + 1) * P],
                            m2_sb[:, k, :],
                            start=(k == 0),
                            stop=(k == MT - 1),
                        )
                    st = stage.tile([P, 512], F16, tag="stage")
                    nc.vector.tensor_copy(st[:], ps[:])
                    nc.sync.dma_start(e2p_v[h][:, me - t0, :], st[:])
                nc.gpsimd.collective_compute(
                    "AllReduce",
                    mybir.AluOpType.add,
                    replica_groups=RG,
                    ins=[e2p_d[h].opt()],
                    outs=[e2r_d[h].opt()],
                )

            e2_sb = persist.tile([P, ET, COUT], F16, tag="slot_e")  # 2MB
            for h, (t0, nt) in enumerate(L2C):
                nc.sync.dma_start(
                    e2_sb[:, t0:t0 + nt, :],
                    e2r_d[h].rearrange("(po pi) c -> pi po c", pi=P),
                )

            # ---- step 6: OUT = sum_e B[e,v] E2[e,c2], [vl, c2] ----
            out_v = out.rearrange("(po pi) c -> pi po c", pi=P)
            for m in range(MT):
                ps = psum_pool.tile([P, 512], F32, tag="ps")
                for k in range(ET):
                    nc.tensor.matmul(
                        ps[:],
                        b_sb[:, k, m * P:(m + 1) * P],
                        e2_sb[:, k, :],
                        start=(k == 0),
                        stop=(k == ET - 1),
                    )
                st = stage.tile([P, 512], F32, tag="stage_out")
                nc.vector.tensor_copy(st[:], ps[:])
                nc.sync.dma_start(out_v[:, m, :], st[:])

    nc.compile()
    return nc


def _prepare_inputs(feature_hyg, pair_v, pair_e, W1, b1, W2, b2, with_bias):
    X = np.ascontiguousarray(np.asarray(feature_hyg, dtype=np.float32))
    pv = np.asarray(pair_v).astype(np.int64)
    pe = np.asarray(pair_e).astype(np.int64)
    W1 = np.asarray(W1, dtype=np.float32)
    b1 = np.asarray(b1, dtype=np.float32)
    W2 = np.asarray(W2, dtype=np.float32)
    b2 = np.asarray(b2, dtype=np.float32)
    ka = KA if with_bias else CIN // P

    ec = np.bincount(pe, minlength=E).astype(np.float32)
    vc = np.bincount(pv, minlength=N).astype(np.float32)
    H = (
        np.bincount(pv * E + pe, minlength=N * E)
        .astype(np.float32)
        .reshape(N, E)
    )
    A = H / np.maximum(ec, 1.0)[None, :]          # [N, E], col-scaled by 1/De
    Bm = (H / np.maximum(vc, 1.0)[:, None]).T     # [E, N], col-scaled by 1/Dv

    W1a = np.zeros((ka * P, CH), np.float16)
    W1a[:CIN] = W1.astype(np.float16)
    W2a = np.zeros((ka * P, COUT), np.float16)
    W2a[:CH] = W2.astype(np.float16)
    if with_bias:
        W1a[CIN] = b1.astype(np.float16)
        W2a[CH] = b2.astype(np.float16)

    in_maps = []
    for l in range(NC):
        sl = slice(l * NL, (l + 1) * NL)
        Xa = np.zeros((ka * P, NL), np.float32)
        Xa[:CIN] = X[sl].T
        if with_bias:
            Xa[CIN] = 1.0
        xt_h = np.ascontiguousarray(
            Xa.reshape(ka, P, MT, P).transpose(2, 1, 0, 3)
        ).reshape(MT, P, ka * P).astype(np.float16)
        a_h = np.ascontiguousarray(
            A[sl].reshape(MT, P, ET, P).transpose(2, 1, 0, 3)
        ).reshape(ET, P, MT * P).astype(np.float16)
        b_h = np.ascontiguousarray(Bm[:, sl]).astype(np.float16)
        in_maps.append(
            {"xt": xt_h, "w1": W1a, "w2": W2a, "a_t": a_h, "b_t": b_h}
        )
    return in_maps


last_result = None  # BassKernelResults of the most recent run (for test harness)


def kernel(feature_hyg, pair_v, pair_e, num_edges, W1, b1, W2, b2):
    global last_result
    assert int(num_edges) == E, f"kernel hardcodes E={E}, got {int(num_edges)}"
    with_bias = bool(np.any(np.asarray(b1)) or np.any(np.asarray(b2)))
    in_maps = _prepare_inputs(
        feature_hyg, pair_v, pair_e, W1, b1, W2, b2, with_bias
    )
    key = ("nc", with_bias)
    if key not in _CACHE:
        _CACHE[key] = _build(with_bias)
    res = run_bass_kernel_spmd(_CACHE[key], in_maps, core_ids=list(range(NC)))
    last_result = res
    out = np.concatenate([res.results[l]["out"] for l in range(NC)], axis=0)
    return np.ascontiguousarray(out.astype(np.float32))
